# revision 19
# baseline (speedup 1.0000x reference)
"""AdaptiveESN Trainium2 kernel — dual fused-chain sequence-split (v4).

Echo State Network: B=64, T=2048, D=128, H=512, leaky a=0.26.
    h_t = (1-a) h_{t-1} + a tanh(x_t W_in^T + b_in + h_{t-1} W_res^T + b_res)
    y_t = h_t W_ro^T

The map is strongly contracting (state error decays ~0.74x/step), so a
chain restarted from h=0 converges to the true trajectory in ~32 steps.

Strategy: 32 overlapping slots of 76 steps (64 useful + 12 washout;
slot 0 starts at t=0 where h=0 is exact, so all its outputs are valid).
Core c runs TWO independent fused 128-lane chains (chain h = slots
4c+2h, 4c+2h+1), interleaved round-robin so one chain's matmuls hide the
other's cross-engine (PE->ACT->DVE->PE) epilogue latency. Per chain-step:
20 matmuls of 128 cols (16 W_res tiles as stationary + 4 W_in), with the
j=3 contraction chunk deferred last in each accumulation group; then 4
per-chunk tanh on ACT (bias via per-partition ACT bias) and 4 fused AXPY
blends on DVE (h' = (1-a) h + p; state h~ = h/a with a folded into
W_res/W_ro so the blend is one scalar_tensor_tensor). Readout (4 steps x
128 lanes per window) is spread ~one window per round between scan steps;
its PSUM->SBUF staging copies all run on DVE (ACT is the busier engine).
The dominant cost on this part is ~50 ns of sync/dispatch overhead per
instruction, so everything is shaped to minimize instruction count at
maximum tile width; PSUM accumulation groups must stay sequential per
region (interleaving groups on one PSUM tile mis-accumulates).

Layouts (host-prepped, per core c; chain h covers slots s=4c+2h (lanes
0-63) and s+1 (lanes 64-127); t_s = 0 for s=0 else 64 s - 12):
    xt   bf16 [128, 2*76*128]  xt[d, (h*76+r)*128+half*64+b] = x[b, t_s+r, d]
    wres bf16 [128, 2048]      tile (j,i) at cols (j*4+i)*128: (a W_res).T block
    win  bf16 [128, 512]       W_in.T
    wro  bf16 [128, 512]       tile j at cols j*128: (a W_ro).T block
    bias f32  [128, 4]         (b_in + b_res) chunk i in col i
    out  f32  [128, 2*76*128]  out[d, (h*76+r)*128+half*64+b] = y[b, t_s+r, d]
Host keeps steps [0,64) of slot 0 and [12,76) of slots s>=1.
"""
import sys

if "/opt/trn_rl_repo" not in sys.path:
    sys.path.insert(0, "/opt/trn_rl_repo")

import numpy as np
import ml_dtypes

import concourse.bass as bass
from concourse import bacc
import concourse.mybir as mybir
import concourse.tile as tile
from concourse.bass_utils import run_bass_kernel_spmd

try:
    import jax

    jax.config.update("jax_compilation_cache_dir", "/tmp/jax_neff_cache")
    jax.config.update("jax_persistent_cache_min_compile_time_secs", 10)
except Exception:
    pass

B, T, D, H = 64, 2048, 128, 512
LEAKY = 0.26
NCORES = 8
NCH = H // 128            # H chunks (partition tiles)
SPC = 4                   # slots per core
NCHAIN = SPC // 2         # fused 128-lane chains per core
NSLOT = NCORES * SPC      # global slots
SEGLEN = T // NSLOT       # stride between slot starts (64)
WO = 12                   # discarded washout steps for slots >= 1
STEPS = SEGLEN + WO       # chain length (76)
LANES = 128               # lanes per fused chain (2 slots x 64 batch)
WST = NCH * LANES         # state cols per step (512)
TCB = 8                   # steps per state buffer
NBUF = 3                  # state buffers per chain
ROW = 4                   # steps per readout window (4*128 = 512 cols)
BF16 = mybir.dt.bfloat16
F32 = mybir.dt.float32

TRACE = False
_last_results = None


def slot_t0(s):
    return 0 if s == 0 else SEGLEN * s - WO


def build(t_total=T, tc=TCB, reps=1, probe=None, fat=False, fatdve=False, rodma=False, psb=6, rob=2, pb=6, ob=3, dvepair=False, rocopy=False):
    """Build the per-core Bacc graph (same graph on all 8 cores).

    reps > 1 wraps the scan in a hardware For_i loop for wall-clock delta
    timing (per-scan = (wall_hi - wall_lo) / (reps_hi - reps_lo)).

    probe: timing-only structural variants (WRONG math, never for output):
      "zrhs"  - scan matmuls read h0 (zero) instead of hprev
      "noro"  - skip readout matmuls/copies/DMAs
      "nodve" - ACT writes states directly (no blend)
    fat=False: per-chunk ACT(+bias)/DVE epilogue (no bias matmuls).
    """
    assert t_total == T, "slot layout is hardcoded for T=2048"
    nc = bacc.Bacc(None, target_bir_lowering=False)
    xt_e = nc.declare_dram_parameter("xt", [128, NCHAIN * STEPS * LANES], BF16, isOutput=False)
    wres_e = nc.declare_dram_parameter("wres", [128, 16 * 128], BF16, isOutput=False)
    win_e = nc.declare_dram_parameter("win", [128, NCH * 128], BF16, isOutput=False)
    wro_e = nc.declare_dram_parameter("wro", [128, NCH * 128], BF16, isOutput=False)
    biasr_e = nc.declare_dram_parameter("biasr", [1, NCH * 128], BF16, isOutput=False)
    bias_e = nc.declare_dram_parameter("bias", [128, NCH], F32, isOutput=False)
    out_e = nc.declare_dram_parameter("out", [128, NCHAIN * STEPS * LANES], F32, isOutput=True)

    with tile.TileContext(nc) as tc_ctx:
        with (
            tc_ctx.tile_pool(name="const", bufs=1) as const_pool,
            tc_ctx.tile_pool(name="p", bufs=pb) as p_pool,
            tc_ctx.tile_pool(name="ostage", bufs=ob) as o_pool,
            tc_ctx.tile_pool(name="scan_ps", bufs=psb, space=bass.MemorySpace.PSUM) as ps_pool,
            tc_ctx.tile_pool(name="ro_ps", bufs=rob, space=bass.MemorySpace.PSUM) as ro_pool,
        ):
            xt_sb = const_pool.tile([128, NCHAIN * STEPS * LANES], BF16)
            wres_sb = const_pool.tile([128, 16 * 128], BF16)
            win_sb = const_pool.tile([128, NCH * 128], BF16)
            wro_sb = const_pool.tile([128, NCH * 128], BF16)
            biasr_sb = const_pool.tile([1, NCH * 128], BF16)
            bias_sb = const_pool.tile([128, NCH], F32)
            ones_sb = const_pool.tile([1, LANES], BF16)
            h0_sb = const_pool.tile([128, WST], BF16)
            # states per chain, step-major: col (r%TCB)*WST + i*LANES + lane
            st = [
                [
                    const_pool.tile([128, TCB * WST], BF16, name=f"st{h}_{n}", tag=f"st{h}_{n}")
                    for n in range(NBUF)
                ]
                for h in range(NCHAIN)
            ]

            nc.sync.dma_start(wres_sb[:], wres_e[:])
            nc.sync.dma_start(win_sb[:], win_e[:])
            nc.sync.dma_start(wro_sb[:], wro_e[:])
            nc.sync.dma_start(biasr_sb[:], biasr_e[:])
            nc.sync.dma_start(bias_sb[:], bias_e[:])
            nc.sync.dma_start(xt_sb[:], xt_e[:])
            nc.vector.memset(ones_sb[:], 1.0)
            nc.vector.memset(h0_sb[:], 0.0)

            def emit_step(h, r):
                if r == 0:
                    hprev = h0_sb[:]
                else:
                    bprev = ((r - 1) // TCB) % NBUF
                    sprev = (r - 1) % TCB
                    hprev = st[h][bprev][:, sprev * WST : (sprev + 1) * WST]
                bcur = (r // TCB) % NBUF
                scur = r % TCB
                xcol = xt_sb[:, (h * STEPS + r) * LANES : (h * STEPS + r + 1) * LANES]
                hsrc = h0_sb[:] if probe == "zrhs" else hprev

                ps = ps_pool.tile([128, WST], F32)

                def psw(i):
                    return ps[:, i * LANES : (i + 1) * LANES]

                def hcol(j):
                    return hsrc[:, j * LANES : (j + 1) * LANES]

                # per-region accumulation groups stay sequential (interleaved
                # groups on one PSUM tile mis-accumulate); (bias, win) first
                # have no state dependency, j=3 deferred last.
                for i in range(NCH):
                    ops = [(win_sb[:, i * 128 : (i + 1) * 128], xcol)]
                    if fat:
                        ops.insert(0, (biasr_sb[:, i * 128 : (i + 1) * 128], ones_sb[:]))
                    ops += [
                        (wres_sb[:, (j * NCH + i) * 128 : (j * NCH + i + 1) * 128], hcol(j))
                        for j in range(NCH)
                    ]
                    for kk, (lhsT, rhs) in enumerate(ops):
                        nc.tensor.matmul(
                            psw(i), lhsT, rhs,
                            start=(kk == 0), stop=(kk == len(ops) - 1))

                st_step = st[h][bcur][:, scur * WST : (scur + 1) * WST]
                if fat:
                    if probe == "nodve":
                        nc.scalar.activation(
                            st_step, ps[:], mybir.ActivationFunctionType.Tanh)
                    else:
                        p_t = p_pool.tile([128, WST], BF16)
                        nc.scalar.activation(
                            p_t[:], ps[:], mybir.ActivationFunctionType.Tanh)
                        nc.vector.scalar_tensor_tensor(
                            st_step, hprev, 1.0 - LEAKY, p_t[:],
                            op0=mybir.AluOpType.mult, op1=mybir.AluOpType.add)
                elif fatdve:
                    # 4 thin tanh (per-chunk bias) into one p tile, 1 AXPY
                    p_t = p_pool.tile([128, WST], BF16)
                    for i in range(NCH):
                        nc.scalar.activation(
                            p_t[:, i * LANES : (i + 1) * LANES], psw(i),
                            mybir.ActivationFunctionType.Tanh,
                            bias=bias_sb[:, i : i + 1])
                    nc.vector.scalar_tensor_tensor(
                        st_step, hprev, 1.0 - LEAKY, p_t[:],
                        op0=mybir.AluOpType.mult, op1=mybir.AluOpType.add)
                elif dvepair:
                    for pair in range(2):
                        p_t = p_pool.tile([128, 2 * LANES], BF16)
                        for k2 in range(2):
                            i = 2 * pair + k2
                            nc.scalar.activation(
                                p_t[:, k2 * LANES : (k2 + 1) * LANES], psw(i),
                                mybir.ActivationFunctionType.Tanh,
                                bias=bias_sb[:, i : i + 1])
                        nc.vector.scalar_tensor_tensor(
                            st_step[:, 2 * pair * LANES : 2 * (pair + 1) * LANES],
                            hprev[:, 2 * pair * LANES : 2 * (pair + 1) * LANES],
                            1.0 - LEAKY, p_t[:],
                            op0=mybir.AluOpType.mult, op1=mybir.AluOpType.add)
                else:
                    for i in range(NCH):
                        st_col = st_step[:, i * LANES : (i + 1) * LANES]
                        bias_ap = bias_sb[:, i : i + 1]
                        if probe == "nodve":
                            nc.scalar.activation(
                                st_col, psw(i), mybir.ActivationFunctionType.Tanh,
                                bias=bias_ap)
                        else:
                            p_t = p_pool.tile([128, LANES], BF16)
                            nc.scalar.activation(
                                p_t[:], psw(i), mybir.ActivationFunctionType.Tanh,
                                bias=bias_ap)
                            nc.vector.scalar_tensor_tensor(
                                st_col,
                                hprev[:, i * LANES : (i + 1) * LANES],
                                1.0 - LEAKY, p_t[:],
                                op0=mybir.AluOpType.mult, op1=mybir.AluOpType.add)

            def emit_ro(h, rs, alt):
                # readout of chain h states for steps [rs, rs+ROW), 128 lanes
                b = (rs // TCB) % NBUF
                ls = rs % TCB
                st_v = st[h][b].rearrange("p (s w) -> p s w", w=WST)
                rps = ro_pool.tile([128, ROW * LANES], F32)
                for j in range(NCH):
                    nc.tensor.matmul(
                        rps[:],
                        wro_sb[:, j * 128 : (j + 1) * 128],
                        st_v[:, ls : ls + ROW, j * LANES : (j + 1) * LANES],
                        start=(j == 0),
                        stop=(j == NCH - 1),
                    )
                dst = out_e[:, (h * STEPS + rs) * LANES : (h * STEPS + rs + ROW) * LANES]
                if rodma:
                    nc.sync.dma_start(dst, rps[:])
                else:
                    ostage = o_pool.tile([128, ROW * LANES], F32)
                    if rocopy is not None:
                        alt = rocopy
                    if alt:
                        nc.scalar.copy(ostage[:], rps[:])
                    else:
                        nc.vector.tensor_copy(ostage[:], rps[:])
                    nc.sync.dma_start(dst, ostage[:])

            def scan_body(_iv=None):
                # windows in production order; one emitted per round
                windows = [
                    (h, rs)
                    for rs in range(0, STEPS, ROW)
                    for h in range(NCHAIN)
                ]
                n_ro = 0
                for r in range(STEPS):
                    for h in range(NCHAIN):
                        emit_step(h, r)
                    if probe == "noro":
                        continue
                    if n_ro < len(windows):
                        h, rs = windows[n_ro]
                        if rs + ROW <= r:  # steps of the window are done
                            emit_ro(h, rs, n_ro % 2 == 0)
                            n_ro += 1
                if probe != "noro":
                    while n_ro < len(windows):
                        h, rs = windows[n_ro]
                        emit_ro(h, rs, n_ro % 2 == 0)
                        n_ro += 1

            if reps == 1:
                scan_body()
            else:
                with tc_ctx.For_i(0, reps, 1) as _i:
                    scan_body(_i)

    nc.compile()
    return nc


def host_prep(x, W_in, b_in, W_res, b_res, W_ro, t_total=T):
    """Produce the per-core in_maps (host-side layout/dtype prep only)."""
    a = np.float32(LEAKY)
    AT = (a * W_res).T.astype(np.float32)                     # [in, out]
    wres = (
        AT.reshape(NCH, 128, NCH, 128).transpose(1, 0, 2, 3).reshape(128, 16 * 128)
    ).astype(ml_dtypes.bfloat16)
    win = W_in.T.astype(ml_dtypes.bfloat16)                   # [128, 512]
    R = (a * W_ro).T.astype(np.float32)                       # [512, 128]
    wro = R.reshape(NCH, 128, 128).transpose(1, 0, 2).reshape(128, NCH * 128).astype(
        ml_dtypes.bfloat16
    )
    bvec = (b_in + b_res).astype(np.float32)
    biasr = bvec.reshape(1, NCH * 128).astype(ml_dtypes.bfloat16)
    bias = bvec.reshape(NCH, 128).T.copy()                    # [128, 4]

    in_maps = []
    for c in range(NCORES):
        xt = np.empty((128, NCHAIN * STEPS * LANES), np.float32)
        xv = xt.reshape(128, NCHAIN, STEPS, 2, B)             # [d, h, r, half, b]
        for h in range(NCHAIN):
            for half in range(2):
                t0 = slot_t0(SPC * c + 2 * h + half)
                xv[:, h, :, half, :] = x[:, t0 : t0 + STEPS, :].transpose(2, 1, 0)
        in_maps.append({
            "xt": xt.astype(ml_dtypes.bfloat16),
            "wres": wres, "win": win, "wro": wro,
            "biasr": biasr, "bias": bias,
        })
    return in_maps


_nc_cache = {}


def kernel(x, W_in, b_in, W_res, b_res, W_ro):
    """Full inputs in, full output out ([B, T, D] float32)."""
    global _last_results
    x, W_in, b_in, W_res, b_res, W_ro = (
        np.asarray(t, dtype=np.float32) for t in (x, W_in, b_in, W_res, b_res, W_ro)
    )
    t_total = x.shape[1]
    if t_total not in _nc_cache:
        _nc_cache[t_total] = build(t_total=t_total)
    nc = _nc_cache[t_total]

    in_maps = host_prep(x, W_in, b_in, W_res, b_res, W_ro, t_total=t_total)
    res = run_bass_kernel_spmd(nc, in_maps, list(range(NCORES)), trace=TRACE)
    _last_results = res

    out = np.empty((B, t_total, D), dtype=np.float32)
    for c in range(NCORES):
        oc = res.results[c]["out"].reshape(128, NCHAIN, STEPS, 2, B)
        for h in range(NCHAIN):
            for half in range(2):
                s = SPC * c + 2 * h + half
                t0 = slot_t0(s)
                u0 = 0 if s == 0 else WO
                out[:, t0 + u0 : t0 + u0 + SEGLEN, :] = (
                    oc[:, h, u0 : u0 + SEGLEN, half, :].transpose(2, 1, 0)
                )
    return out
